# revision 19
# baseline (speedup 1.0000x reference)
"""Trainium2 Bass kernel for nn_AttentionLayer_83545703842160.

Single-head attention over spatial tokens, per batch element:
  t = x[b].reshape(C, H*W).T            # [N, C], N=4096, C=64
  q,k,v = t@W{q,k,v}.T + b{q,k,v}
  out   = softmax(q@k.T / sqrt(C)) @ v  # -> [C, N] -> [C, H, W]

Sharding: data-parallel over batch B=8 across the 8 NeuronCores (one
batch element per core). Each core holds the full (tiny) QKV weights.

Per-core kernel, v3 (bf16 streaming end-to-end):
  - xw [65, 256+4096] bf16: packed [w | x] so ONE head DMA lands both the
    QKV weights and the first x chunk. x is in [C, N] layout with a
    host-appended ones row so biases fold into the contraction (K = 65);
    w cols = [wq | wk | wv_ext]. Host casts to bf16 (halves input DMA).
  - qT,kT [64, 4096] bf16 = W{q,k}_ext @ xt (PE), PSUM->SBUF by DVE (the
    very first q chunk by ACT, which idles until the first exp).
  - v_sb [128, 32, 66] bf16 token-major v with a ones column (col 64, the
    softmax denominator) and a zero pad column; produced in chunks of 4
    m-tiles per PSUM slot (8 slots instead of 32 -> less ring pressure).
  - one global stream of 88 score groups (8 superblocks x 11 m-tile
    groups; [2,3x10], reversed for the last superblock so the final exp
    is short). Group g: MM1s (kT-tile.T @ q-chunk -> PSUM fp32)
    interleaved with stage-2 MM2 groups (v_ext.T @ exp'd scores,
    accumulating [66, S] in PSUM; row 64 = denominator via ones column)
    and projection producers. exp(0.125*scores) drains each group
    PSUM->SBUF bf16 on ACT -- the bottleneck engine (N^2 = 16.7M
    elems/core at 1 elem/lane/cycle @ 1.2 GHz: ~110 us floor, ~126 us
    with per-instruction bubbles).
  - stage-2 starts at group 13 (so the projection-heavy opening groups
    carry no MM2 load) and catches up to a lag of 3 groups by the end via
    10 spread-out double-MM2 groups; only ~2 MM2 groups + one tail remain
    after the last exp (v1 exposed a full superblock: 32 MM2s).
  - tail per superblock: reciprocal(denominator row) read directly from
    PSUM, gpsimd partition_broadcast, DVE multiply -> bf16 out, DMA out.
    The final tail broadcasts via a tiny PE matmul (ones.T @ recip) into a
    then-free scores PSUM slot instead of gpsimd. Host upcasts y -> fp32.
  PSUM: scores ping-pong 2x3 banks + acc/projection pool 2x1 = 8 banks.

Measured per-body on HW (1-core reps-loop delta, R=51 vs 11): see
_hw_time_ns.txt; harness-reported baseline (v1) was 175038 ns.
"""

import numpy as np
from contextlib import ExitStack

import ml_dtypes

import concourse.bacc as bacc
import concourse.bass as bass
import concourse.mybir as mybir
import concourse.tile as tile
from concourse.bass import MemorySpace
from concourse.bass_utils import run_bass_kernel_spmd

C = 64          # channels
N = 4096        # tokens (64*64 spatial)
B = 8           # batch == number of cores
S = 512         # query superblock
MT = 128        # keys per m-tile
NMT = N // MT   # 32 m-tiles
WPAD = 256      # xw columns reserved for the packed weights
WCOLS = 2 * C + C + 2   # packed weight tensor: [wq | wk | wv_ext]
FP32 = mybir.dt.float32
F32R = mybir.dt.float32r
BF16 = mybir.dt.bfloat16
EXP = mybir.ActivationFunctionType.Exp
NSB = N // S                # 8 superblocks
GROUPS = [2] + [3] * 10     # m-tiles per exp group within a superblock
NGRP = len(GROUPS)          # 11 groups per superblock
NG = NSB * NGRP             # 88 global groups
S2START = 13                # first global group that carries stage-2 work
S2EXTRA = (20, 27, 34, 41, 48, 55, 62, 69, 76, 83)  # double-MM2 groups


def _ginfo(g):
    """global group -> (superblock, m-tile base, group size). The last
    superblock runs its groups reversed ([3]*10+[2]) so the final exp
    instruction is the short one."""
    s, gi = divmod(g, NGRP)
    if s == NSB - 1:
        gi = NGRP - 1 - gi
    return s, sum(GROUPS[:gi]), GROUPS[gi]


def _build_kernel(tc, ctx, xw_d, y_d, reps=1):
    if reps > 1:
        # timing harness: repeat the whole body in a HW loop so kernel time
        # dominates dispatch overhead in wallclock measurements
        engines = (mybir.EngineType.PE, mybir.EngineType.Activation,
                   mybir.EngineType.DVE, mybir.EngineType.Pool,
                   mybir.EngineType.SP)
        with tc.For_i(0, reps, 1, hint_engines=engines):
            _build_body(tc, ctx, xw_d, y_d)
    else:
        _build_body(tc, ctx, xw_d, y_d)


def _build_body(tc, ctx, xw_d, y_d):
    nc = tc.nc

    sb = ctx.enter_context(tc.tile_pool(name="sb", bufs=1))
    pt_pool = ctx.enter_context(tc.tile_pool(name="pt", bufs=S2START + 1))
    osb_pool = ctx.enter_context(tc.tile_pool(name="osb", bufs=2))
    nrm_pool = ctx.enter_context(tc.tile_pool(name="nrm", bufs=2))
    sc_psum = ctx.enter_context(
        tc.tile_pool(name="scp", bufs=2, space=MemorySpace.PSUM))
    ac_psum = ctx.enter_context(
        tc.tile_pool(name="acp", bufs=2, space=MemorySpace.PSUM))

    xw = sb.tile([C + 1, WPAD + N], BF16)
    qt = sb.tile([C, N], BF16)
    kt = sb.tile([C, N], BF16)
    v_sb = sb.tile([MT, NMT, C + 2], BF16)

    xt = xw[:, WPAD:WPAD + N]
    wq = xw[:, 0:C]
    wk = xw[:, C:2 * C]
    wv = xw[:, 2 * C:WCOLS]

    # One head DMA lands w + the first x chunk (sync queue); the rest of x
    # streams in on gpsimd's queue in three descriptors sized so each lands
    # before the first projection needing it (k1 at group 0, k2-3 at groups
    # 1-2, the rest later).
    nc.sync.dma_start(xw[:, 0:WPAD + S], xw_d[:, 0:WPAD + S])
    for lo, hi in ((S, 2 * S), (2 * S, 5 * S), (5 * S, N)):
        nc.gpsimd.dma_start(xw[:, WPAD + lo:WPAD + hi],
                            xw_d[:, WPAD + lo:WPAD + hi])

    # Projection producers, emitted piecemeal between matmuls so PSUM-slot
    # and DVE waits hide under other PE work.
    def emit_qk(w_slice, dst, j, on_act=False, split_copy=False):
        p = ac_psum.tile([C, S], FP32, tag="ps1")
        nc.tensor.matmul(p[:], w_slice, xt[:, j * S:(j + 1) * S],
                         start=True, stop=True)
        if on_act:
            nc.scalar.copy(dst[:, j * S:(j + 1) * S], p[:])
        elif split_copy:
            # halves, so the first MM1s (which only need the low half) start
            # one half-copy earlier
            h = S // 2
            nc.vector.tensor_copy(dst[:, j * S:j * S + h], p[:, 0:h])
            nc.vector.tensor_copy(dst[:, j * S + h:(j + 1) * S], p[:, h:S])
        else:
            nc.vector.tensor_copy(dst[:, j * S:(j + 1) * S], p[:])

    def emit_v4(c):
        # 4 m-tiles' worth of v in one PSUM slot / one DVE copy
        p = ac_psum.tile([MT, 4, C + 2], FP32, tag="ps1")
        for i in range(4):
            m = 4 * c + i
            nc.tensor.matmul(p[:, i, :], xt[:, m * MT:(m + 1) * MT], wv,
                             start=True, stop=True)
        nc.vector.tensor_copy(v_sb[:, 4 * c:4 * c + 4, :], p[:])

    def emit_tail(acc, s, final):
        # normalize: y[:, block] = acc[0:64] / acc[64] (denominator row)
        if not final:
            rr = nrm_pool.tile([1, S], F32R, tag="rr")
            # f32r is bit-identical to fp32; the tag only steers the PE
            # matmul broadcast of the final tail onto the fast f32r path
            with nc.allow_low_precision(reason="f32r == fp32 bits"):
                nc.vector.reciprocal(rr[:], acc[C:C + 1, :])
            bc = nrm_pool.tile([C, S], F32R, tag="bc")
            nc.gpsimd.partition_broadcast(bc[:], rr[:], channels=C)
            ob = osb_pool.tile([C, S], BF16, tag="ob")
            nc.vector.tensor_mul(ob[:], acc[0:C, :], bc[:])
            nc.sync.dma_start(y_d[:, s * S:(s + 1) * S], ob[:])
            return
        # final superblock: this chain is fully exposed at the kernel end, so
        # pipeline it in halves (recip/broadcast/mul/DMA overlap across
        # halves); both reciprocals up front so DVE never idles.
        HS = S // 2
        rrs = []
        for half in range(2):
            rr = nrm_pool.tile([1, HS], F32R, tag="rr")
            with nc.allow_low_precision(reason="f32r == fp32 bits"):
                nc.vector.reciprocal(
                    rr[:], acc[C:C + 1, half * HS:(half + 1) * HS])
            rrs.append(rr)
        for half in range(2):
            lo = half * HS
            bc = nrm_pool.tile([C, HS], F32R, tag="bc")
            nc.gpsimd.partition_broadcast(bc[:], rrs[half][:], channels=C)
            ob = osb_pool.tile([C, HS], BF16, tag="ob")
            nc.vector.tensor_mul(ob[:], acc[0:C, lo:lo + HS], bc[:])
            nc.sync.dma_start(y_d[:, s * S + lo:s * S + lo + HS], ob[:])

    # producer schedule: thunk lists keyed by global group.
    # k chunk c lands during group c-1 (first MM1 needing it is in group
    # ~4c/3); v chunks of 4 m-tiles land in groups 7..14 (first MM2 needing
    # chunk c runs at group >= S2START + 4c/3); q chunk j mid-superblock j-1.
    producers = {g: [] for g in range(NG)}
    for c in range(1, NSB):
        producers[c - 1].append(lambda c=c: emit_qk(wk, kt, c))
    for c in range(NSB):
        producers[7 + c].append(lambda c=c: emit_v4(c))
    producers[8].append(lambda: emit_qk(wq, qt, 1))
    for j in range(2, NSB):
        producers[NGRP * (j - 1) + 2].append(lambda j=j: emit_qk(wq, qt, j))

    # stage-2 schedule: which stage-2 groups run inside global group g
    s2sched = {g: [] for g in range(NG)}
    h = 0
    for g in range(S2START, NG):
        s2sched[g].append(h)
        h += 1
        if g in S2EXTRA:
            s2sched[g].append(h)
            h += 1
    s2_drain = list(range(h, NG))

    state = {"acc": None}
    pts = {}

    def mm2_thunks(h):
        s2, m0, gs2 = _ginfo(h)
        thunks = []
        if h % NGRP == 0:
            def alloc():
                state["acc"] = ac_psum.tile([C + 2, S], FP32, tag="ps1",
                                            name="acc")
            thunks.append(alloc)
        for j in range(gs2):
            def mm2(j=j, m0=m0, h=h, gs2=gs2):
                # start/stop follow execution order (the last superblock's
                # groups run reversed), not the m-tile index
                nc.tensor.matmul(
                    state["acc"][:], v_sb[:, m0 + j, :],
                    pts[h][:, j * S:(j + 1) * S],
                    start=(h % NGRP == 0 and j == 0),
                    stop=(h % NGRP == NGRP - 1 and j == gs2 - 1))
            thunks.append(mm2)
        if h % NGRP == NGRP - 1:
            def tail(s2=s2, h=h):
                emit_tail(state["acc"], s2, final=(s2 == NSB - 1))
                del pts[h]
            thunks.append(tail)
        return thunks

    # head: only what the very first scores group needs. ACT (idle until
    # the first exp) does the q-chunk-0 PSUM->SBUF copy in parallel with
    # DVE's k-chunk-0 copy.
    emit_qk(wq, qt, 0, on_act=True)
    emit_qk(wk, kt, 0, split_copy=True)

    for g in range(NG):
        s, m0, gs = _ginfo(g)
        qs = qt[:, s * S:(s + 1) * S]
        sc = sc_psum.tile([MT, gs * S], FP32, tag="sc")
        extra = []
        for h2 in s2sched[g]:
            extra.extend(mm2_thunks(h2))
        extra.extend(producers[g])
        for j in range(gs):
            nc.tensor.matmul(
                sc[:, j * S:(j + 1) * S],
                kt[:, (m0 + j) * MT:(m0 + j + 1) * MT], qs,
                start=True, stop=True)
            for t in extra[3 * j:3 * (j + 1)]:
                t()
        for t in extra[3 * gs:]:
            t()
        pt = pt_pool.tile([MT, gs * S], BF16, tag="pt")
        nc.scalar.activation(pt[:], sc[:], EXP, scale=0.125)
        pts[g] = pt
    for h2 in s2_drain:
        for t in mm2_thunks(h2):
            t()


_NC_CACHE = {}


def _get_nc(reps=1):
    if reps not in _NC_CACHE:
        nc = bacc.Bacc("TRN2", target_bir_lowering=False, debug=False,
                       enable_asserts=False)
        xw_d = nc.dram_tensor("xw", [C + 1, WPAD + N], BF16,
                              kind="ExternalInput").ap()
        y_d = nc.dram_tensor("y", [C, N], BF16, kind="ExternalOutput").ap()
        with tile.TileContext(nc) as tc:
            with ExitStack() as ctx:
                _build_kernel(tc, ctx, xw_d, y_d, reps=reps)
        nc.compile()
        _NC_CACHE[reps] = nc
    return _NC_CACHE[reps]


def _host_weights(Wq, bq, Wk, bk, Wv, bv):
    w = np.zeros((C + 1, WPAD), np.float32)
    w[:C, 0:C] = np.asarray(Wq, np.float32).T
    w[C, 0:C] = bq
    w[:C, C:2 * C] = np.asarray(Wk, np.float32).T
    w[C, C:2 * C] = bk
    w[:C, 2 * C:3 * C] = np.asarray(Wv, np.float32).T
    w[C, 2 * C:3 * C] = bv
    w[C, 3 * C] = 1.0  # ones column of v_ext; col 3C+1 stays zero padding
    return w


def _host_xw(x_b, w):
    xw = np.concatenate(
        [w, np.concatenate([np.asarray(x_b, np.float32).reshape(C, N),
                            np.ones((1, N), np.float32)], axis=0)], axis=1)
    return np.ascontiguousarray(xw.astype(ml_dtypes.bfloat16))


def _in_maps(inputs):
    x = np.asarray(inputs["x"], np.float32)
    w = _host_weights(inputs["Wq"], inputs["bq"], inputs["Wk"],
                      inputs["bk"], inputs["Wv"], inputs["bv"])
    return [{"xw": _host_xw(x[b], w)} for b in range(B)]


def _run(inputs, reps=1, **spmd_kwargs):
    nc = _get_nc(reps)
    in_maps = _in_maps(inputs)
    res = run_bass_kernel_spmd(nc, in_maps, core_ids=list(range(B)),
                               **spmd_kwargs)
    out = np.stack([np.asarray(res.results[b]["y"], np.float32)
                    .reshape(C, 64, 64) for b in range(B)], axis=0)
    return out, res


def kernel(**inputs):
    out, _ = _run(inputs)
    return out


# revision 24
# speedup vs baseline: 1.4843x; 1.4843x over previous
"""Trainium2 Bass kernel for nn_AttentionLayer_83545703842160.

Single-head attention over spatial tokens, per batch element:
  t = x[b].reshape(C, H*W).T            # [N, C], N=4096, C=64
  q,k,v = t@W{q,k,v}.T + b{q,k,v}
  out   = softmax(q@k.T / sqrt(C)) @ v  # -> [C, N] -> [C, H, W]

Sharding: data-parallel over batch B=8 across the 8 NeuronCores (one
batch element per core). Each core holds the full (tiny) QKV weights.

Per-core kernel, v4 (bf16 streaming end-to-end):
  - xw [65, 256+4096] bf16: packed [w | x] so ONE head DMA lands both the
    QKV weights and the first x chunk. x is in [C, N] layout with a
    host-appended ones row so biases fold into the contraction (K = 65);
    w cols = [wq | wk | wv_ext]. Host casts to bf16 (halves input DMA);
    y returns bf16 and is upcast to fp32 on host (halves output DMA).
  - qT,kT [64, 4096] bf16 = W{q,k}_ext @ xt (PE), PSUM->SBUF by DVE (the
    very first q chunk by ACT, which idles until the first exp).
  - v_sb [128, 32, 66] bf16 token-major v with a ones column (col 64, the
    softmax denominator) and a zero pad column; produced in chunks of 4
    m-tiles per PSUM slot (8 slots instead of 32 -> less ring pressure).
  - one global stream of 88 score groups (8 superblocks x 11 m-tile
    groups; [2,3x10], reversed for the last superblock so the final exp
    is the short one). Group g: MM1s (kT-tile.T @ q-chunk -> PSUM fp32)
    interleaved with stage-2 MM2 groups (v_ext.T @ exp'd scores,
    accumulating [66, S] in PSUM; row 64 = denominator via ones column)
    and projection producers. exp(0.125*scores) drains each group
    PSUM->SBUF bf16 on ACT -- the bottleneck engine (N^2 = 16.7M
    elems/core at 1 elem/lane/cycle @ 1.2 GHz: ~110 us floor, ~126 us
    with per-instruction bubbles; the 8-bank PSUM budget pins the chunk
    size at 3 m-tiles, so ~88 ACTIVATEs is forced).
  - stage-2 starts at group 15 (the projection-heavy opening groups carry
    no MM2 load, mirroring why v1 lagged by a full superblock) and
    catches up to a lag of 3 groups by the end via 12 spread-out
    double-MM2 groups; only 3 short MM2 groups + one tail remain after
    the last exp (v1 exposed a full superblock: 32 MM2s).
  - tail per superblock: reciprocal(denominator row) read directly from
    PSUM by DVE, gpsimd partition_broadcast, DVE multiply -> bf16, DMA
    out. Host upcasts y -> fp32. (Tried and rejected: PE-matmul
    broadcast -- DVE cannot read two PSUM operands on HW; halved final
    tail -- cross-engine hop latency eats the gain.)
  PSUM: scores ping-pong 2x3 banks + acc/projection pool 2x1 = 8 banks.

TimelineSim: v1 142.5us -> v4 138.1us; the same model/HW ratio as v1
(142.5 model / 175.0us harness) predicts ~170us. On top of that, v4
halves DMA bytes and drops ~90 instructions (fewer DVE copies, fewer
PSUM-ring waits), which the model does not credit. _hw_time_ns.txt
holds a reps-loop steady-state throughput number (see time_hw.py) --
an upper bound, not comparable to the harness's single-shot NTFF time.
"""

import numpy as np
from contextlib import ExitStack

import ml_dtypes

import concourse.bacc as bacc
import concourse.bass as bass
import concourse.mybir as mybir
import concourse.tile as tile
from concourse.bass import MemorySpace
from concourse.bass_utils import run_bass_kernel_spmd

C = 64          # channels
N = 4096        # tokens (64*64 spatial)
B = 8           # batch == number of cores
S = 512         # query superblock
MT = 128        # keys per m-tile
NMT = N // MT   # 32 m-tiles
WPAD = 256      # xw columns reserved for the packed weights
WCOLS = 2 * C + C + 2   # packed weight tensor: [wq | wk | wv_ext]
FP32 = mybir.dt.float32
F32R = mybir.dt.float32r
BF16 = mybir.dt.bfloat16
EXP = mybir.ActivationFunctionType.Exp
NSB = N // S                # 8 superblocks
GROUPS = [2] + [3] * 10     # m-tiles per exp group within a superblock
NGRP = len(GROUPS)          # 11 groups per superblock
NG = NSB * NGRP             # 88 global groups
S2START = 15                # first global group that carries stage-2 work
S2EXTRA = (20, 26, 32, 38, 44, 50, 56, 62, 68, 74, 80, 84)  # double-MM2 groups


def _ginfo(g):
    """global group -> (superblock, m-tile base, group size). The last
    superblock runs its groups reversed ([3]*10+[2]) so the final exp
    instruction is the short one."""
    s, gi = divmod(g, NGRP)
    if s == NSB - 1:
        gi = NGRP - 1 - gi
    return s, sum(GROUPS[:gi]), GROUPS[gi]


def _build_kernel(tc, ctx, xw_d, y_d, reps=1):
    if reps > 1:
        # timing harness: repeat the whole body in a HW loop so kernel time
        # dominates dispatch overhead in wallclock measurements
        engines = (mybir.EngineType.PE, mybir.EngineType.Activation,
                   mybir.EngineType.DVE, mybir.EngineType.Pool,
                   mybir.EngineType.SP)
        with tc.For_i(0, reps, 1, hint_engines=engines):
            _build_body(tc, ctx, xw_d, y_d)
    else:
        _build_body(tc, ctx, xw_d, y_d)


def _build_body(tc, ctx, xw_d, y_d):
    nc = tc.nc

    sb = ctx.enter_context(tc.tile_pool(name="sb", bufs=1))
    pt_pool = ctx.enter_context(tc.tile_pool(name="pt", bufs=S2START + 1))
    osb_pool = ctx.enter_context(tc.tile_pool(name="osb", bufs=2))
    nrm_pool = ctx.enter_context(tc.tile_pool(name="nrm", bufs=2))
    sc_psum = ctx.enter_context(
        tc.tile_pool(name="scp", bufs=2, space=MemorySpace.PSUM))
    ac_psum = ctx.enter_context(
        tc.tile_pool(name="acp", bufs=2, space=MemorySpace.PSUM))

    xw = sb.tile([C + 1, WPAD + N], BF16)
    qt = sb.tile([C, N], BF16)
    kt = sb.tile([C, N], BF16)
    v_sb = sb.tile([MT, NMT, C + 2], BF16)

    xt = xw[:, WPAD:WPAD + N]
    wq = xw[:, 0:C]
    wk = xw[:, C:2 * C]
    wv = xw[:, 2 * C:WCOLS]

    # One head DMA lands w + the first x chunk (sync queue); the rest of x
    # streams in on gpsimd's queue in three descriptors sized so each lands
    # before the first projection needing it (k1 at group 0, k2-3 at groups
    # 1-2, the rest later).
    nc.sync.dma_start(xw[:, 0:WPAD + S], xw_d[:, 0:WPAD + S])
    for lo, hi in ((S, 2 * S), (2 * S, 5 * S), (5 * S, N)):
        nc.gpsimd.dma_start(xw[:, WPAD + lo:WPAD + hi],
                            xw_d[:, WPAD + lo:WPAD + hi])

    # Projection producers, emitted piecemeal between matmuls so PSUM-slot
    # and DVE waits hide under other PE work.
    def emit_qk(w_slice, dst, j, on_act=False, split_copy=False):
        p = ac_psum.tile([C, S], FP32, tag="ps1")
        nc.tensor.matmul(p[:], w_slice, xt[:, j * S:(j + 1) * S],
                         start=True, stop=True)
        if on_act:
            nc.scalar.copy(dst[:, j * S:(j + 1) * S], p[:])
        elif split_copy:
            # halves, so the first MM1s (which only need the low half) start
            # one half-copy earlier
            h = S // 2
            nc.vector.tensor_copy(dst[:, j * S:j * S + h], p[:, 0:h])
            nc.vector.tensor_copy(dst[:, j * S + h:(j + 1) * S], p[:, h:S])
        else:
            nc.vector.tensor_copy(dst[:, j * S:(j + 1) * S], p[:])

    def emit_v4(c):
        # 4 m-tiles' worth of v in one PSUM slot / one DVE copy
        p = ac_psum.tile([MT, 4, C + 2], FP32, tag="ps1")
        for i in range(4):
            m = 4 * c + i
            nc.tensor.matmul(p[:, i, :], xt[:, m * MT:(m + 1) * MT], wv,
                             start=True, stop=True)
        nc.vector.tensor_copy(v_sb[:, 4 * c:4 * c + 4, :], p[:])

    def emit_tail(acc, s, final):
        # normalize: y[:, block] = acc[0:64] / acc[64] (denominator row)
        if not final:
            rr = nrm_pool.tile([1, S], F32R, tag="rr")
            # f32r is bit-identical to fp32; the tag only steers the PE
            # matmul broadcast of the final tail onto the fast f32r path
            with nc.allow_low_precision(reason="f32r == fp32 bits"):
                nc.vector.reciprocal(rr[:], acc[C:C + 1, :])
            bc = nrm_pool.tile([C, S], F32R, tag="bc")
            nc.gpsimd.partition_broadcast(bc[:], rr[:], channels=C)
            ob = osb_pool.tile([C, S], BF16, tag="ob")
            nc.vector.tensor_mul(ob[:], acc[0:C, :], bc[:])
            nc.sync.dma_start(y_d[:, s * S:(s + 1) * S], ob[:])
            return
        # final superblock: same single chain (cross-engine hop latency beats
        # any compute saved by splitting into halves)
        rr = nrm_pool.tile([1, S], F32R, tag="rr")
        with nc.allow_low_precision(reason="f32r == fp32 bits"):
            nc.vector.reciprocal(rr[:], acc[C:C + 1, :])
        bc = nrm_pool.tile([C, S], F32R, tag="bc")
        nc.gpsimd.partition_broadcast(bc[:], rr[:], channels=C)
        ob = osb_pool.tile([C, S], BF16, tag="ob")
        nc.vector.tensor_mul(ob[:], acc[0:C, :], bc[:])
        nc.sync.dma_start(y_d[:, s * S:(s + 1) * S], ob[:])

    # producer schedule: thunk lists keyed by global group.
    # k chunk c lands during group c-1 (first MM1 needing it is in group
    # ~4c/3); v chunks of 4 m-tiles land in groups 7..14 (first MM2 needing
    # chunk c runs at group >= S2START + 4c/3); q chunk j mid-superblock j-1.
    producers = {g: [] for g in range(NG)}
    for c in range(1, NSB):
        producers[c - 1].append(lambda c=c: emit_qk(wk, kt, c))
    producers[7].append(lambda: emit_qk(wq, qt, 1))
    for c in range(NSB):
        producers[8 + c].append(lambda c=c: emit_v4(c))
    # q chunks j>=2: groups picked to dodge the double-MM2 catch-up groups
    # and v-chunk groups; deadline for chunk j is group 11*j
    for j, g in zip(range(2, NSB), (17, 23, 35, 47, 59, 71)):
        producers[g].append(lambda j=j: emit_qk(wq, qt, j))

    # stage-2 schedule: which stage-2 groups run inside global group g
    s2sched = {g: [] for g in range(NG)}
    h = 0
    for g in range(S2START, NG):
        s2sched[g].append(h)
        h += 1
        if g in S2EXTRA:
            s2sched[g].append(h)
            h += 1
    s2_drain = list(range(h, NG))

    state = {"acc": None}
    pts = {}

    def mm2_thunks(h):
        s2, m0, gs2 = _ginfo(h)
        thunks = []
        if h % NGRP == 0:
            def alloc():
                state["acc"] = ac_psum.tile([C + 2, S], FP32, tag="ps1",
                                            name="acc")
            thunks.append(alloc)
        for j in range(gs2):
            def mm2(j=j, m0=m0, h=h, gs2=gs2):
                # start/stop follow execution order (the last superblock's
                # groups run reversed), not the m-tile index
                nc.tensor.matmul(
                    state["acc"][:], v_sb[:, m0 + j, :],
                    pts[h][:, j * S:(j + 1) * S],
                    start=(h % NGRP == 0 and j == 0),
                    stop=(h % NGRP == NGRP - 1 and j == gs2 - 1))
            thunks.append(mm2)
        if h % NGRP == NGRP - 1:
            def tail(s2=s2, h=h):
                emit_tail(state["acc"], s2, final=(s2 == NSB - 1))
                del pts[h]
            thunks.append(tail)
        return thunks

    # head: only what the very first scores group needs. ACT (idle until
    # the first exp) does the q-chunk-0 PSUM->SBUF copy in parallel with
    # DVE's k-chunk-0 copy.
    emit_qk(wq, qt, 0, on_act=True)
    emit_qk(wk, kt, 0, split_copy=True)

    for g in range(NG):
        s, m0, gs = _ginfo(g)
        qs = qt[:, s * S:(s + 1) * S]
        sc = sc_psum.tile([MT, gs * S], FP32, tag="sc")
        extra = []
        for h2 in s2sched[g]:
            extra.extend(mm2_thunks(h2))
        extra.extend(producers[g])
        for j in range(gs):
            nc.tensor.matmul(
                sc[:, j * S:(j + 1) * S],
                kt[:, (m0 + j) * MT:(m0 + j + 1) * MT], qs,
                start=True, stop=True)
            for t in extra[3 * j:3 * (j + 1)]:
                t()
        for t in extra[3 * gs:]:
            t()
        pt = pt_pool.tile([MT, gs * S], BF16, tag="pt")
        nc.scalar.activation(pt[:], sc[:], EXP, scale=0.125)
        pts[g] = pt
    for h2 in s2_drain:
        for t in mm2_thunks(h2):
            t()


_NC_CACHE = {}


def _get_nc(reps=1):
    if reps not in _NC_CACHE:
        nc = bacc.Bacc("TRN2", target_bir_lowering=False, debug=False,
                       enable_asserts=False)
        xw_d = nc.dram_tensor("xw", [C + 1, WPAD + N], BF16,
                              kind="ExternalInput").ap()
        y_d = nc.dram_tensor("y", [C, N], BF16, kind="ExternalOutput").ap()
        with tile.TileContext(nc) as tc:
            with ExitStack() as ctx:
                _build_kernel(tc, ctx, xw_d, y_d, reps=reps)
        nc.compile()
        _NC_CACHE[reps] = nc
    return _NC_CACHE[reps]


def _host_weights(Wq, bq, Wk, bk, Wv, bv):
    w = np.zeros((C + 1, WPAD), np.float32)
    w[:C, 0:C] = np.asarray(Wq, np.float32).T
    w[C, 0:C] = bq
    w[:C, C:2 * C] = np.asarray(Wk, np.float32).T
    w[C, C:2 * C] = bk
    w[:C, 2 * C:3 * C] = np.asarray(Wv, np.float32).T
    w[C, 2 * C:3 * C] = bv
    w[C, 3 * C] = 1.0  # ones column of v_ext; col 3C+1 stays zero padding
    return w


def _host_xw(x_b, w):
    xw = np.concatenate(
        [w, np.concatenate([np.asarray(x_b, np.float32).reshape(C, N),
                            np.ones((1, N), np.float32)], axis=0)], axis=1)
    return np.ascontiguousarray(xw.astype(ml_dtypes.bfloat16))


def _in_maps(inputs):
    x = np.asarray(inputs["x"], np.float32)
    w = _host_weights(inputs["Wq"], inputs["bq"], inputs["Wk"],
                      inputs["bk"], inputs["Wv"], inputs["bv"])
    return [{"xw": _host_xw(x[b], w)} for b in range(B)]


def _run(inputs, reps=1, **spmd_kwargs):
    nc = _get_nc(reps)
    in_maps = _in_maps(inputs)
    res = run_bass_kernel_spmd(nc, in_maps, core_ids=list(range(B)),
                               **spmd_kwargs)
    out = np.stack([np.asarray(res.results[b]["y"], np.float32)
                    .reshape(C, 64, 64) for b in range(B)], axis=0)
    return out, res


def kernel(**inputs):
    out, _ = _run(inputs)
    return out
